# revision 1
# baseline (speedup 1.0000x reference)
"""Event-driven FFN kernel for Trainium2 (8 NeuronCores, data-parallel).

Reference computation (per row r of x[32768, 512]):
    mask[r] = any(|x[r, :]| > 0.01)
    y[r, :] = mask[r] * (relu(x[r, :] @ w1 + b1) @ w2 + b2)

Sharding: rows (B*T*S = 32768) split evenly across 8 cores; FFN weights
replicated.  Per core: 4096 rows, processed in 8 blocks of 512 rows.

Per-block dataflow on one core (512 rows per block):
  - DMA x block natural layout [128p, 4rs, 512d]
  - abs-max over d per row -> spike mask (VectorE reduce + is_gt)
  - PE transpose (identity matmul, f32r) -> xT [128d_in, 4dc, 512r]
  - mm1 per f-chunk (16): psum_h[f,r] += w1[dc,f].T @ xT[dc,r]  (4 MMs)
    ReLU+b1 on ScalarE -> hT sbuf [128f_in, 16fc, 512r] (f32r, rounded)
  - mm2 two f-chunks behind mm1 (software pipeline): psum_y[rt] +=
    hT[:,fc,rt].T @ w2[fc,:] -> natural-layout y rows in PSUM (4 banks
    live across the f loop; 5-slot pool so slot reuse never stalls PE)
  - epilogue: yb = b2*mask precomputed off-path; one fused VectorE op per
    row-subtile yout = psy*mask + yb, then DMA out per row-subtile

Scheduling notes:
  - Block b+1's x load + PE transposes are emitted mid-way through block
    b's f-loop so PE never stalls on the DVE xT copies.
  - Weights stream in chunks in first-use order; block 0 defers all mm2s
    past its mm1 phase so they aren't gated on the still-streaming w2.
  - All matmuls use float32r (PE truncates to ~FP22, full 1 cycle/row
    streaming rate; plain float32 runs 4 passes = 4x slower).  Rel err vs
    the f32 reference is ~2e-4.
  - Built on bacc.Bacc: finalize() legalizes multi-sem-wait instructions
    (TRN2 engines accept one sem wait per instruction).

TimelineSim (cost model): ~241.9 us/core end-to-end one-shot; steady-state
marginal pass is ~228.8 us = PE fully saturated (218 us matmuls at the
1 cycle/row f32r rate + 10 us PE transposes).  Remaining one-shot overhead
is the DMA-bandwidth-bound startup stream (~3 us; 2 MB must land before
the first mm1 can run), the Tile entry barrier (~1.4 us) and exit drain
(~4.3 us; the last block groups mm2 by row-subtile so only one epilogue
trails the final matmul).  PSUM: 3 staging banks (transpose + mm1
accumulator, shared tag) + 4 y-accumulator banks + 1 warmup bank.
"""

import numpy as np

N_CORES = 8
ROWS_TOTAL = 32768  # 4 * 16 * 512
ROWS_PER_CORE = ROWS_TOTAL // N_CORES  # 4096
D = 512
F = 2048
R_BLOCK = 512
N_BLOCKS = ROWS_PER_CORE // R_BLOCK  # 8
P = 128
DC = D // P  # 4 d-chunks
FC = F // P  # 16 f-chunks
RT = R_BLOCK // P  # 4 row-subtiles per block
THRESHOLD = 0.01

_CACHE = {}


def _build_program(repeat=1):
    import concourse.mybir as mybir
    import concourse.tile as tile
    from concourse import bacc
    from concourse.masks import make_identity

    f32 = mybir.dt.float32
    f32r = mybir.dt.float32r
    # Bacc (not plain Bass): finalize() runs the wait-splitting legalization
    # (generate_event_semaphores) required by TRN2's 1-wait-per-instruction
    # hardware limit.
    nc = bacc.Bacc()

    x = nc.declare_dram_parameter("x", [ROWS_PER_CORE, D], f32, isOutput=False)
    w1 = nc.declare_dram_parameter("w1", [D, F], f32, isOutput=False)
    b1 = nc.declare_dram_parameter("b1", [F], f32, isOutput=False)
    w2 = nc.declare_dram_parameter("w2", [F, D], f32, isOutput=False)
    b2 = nc.declare_dram_parameter("b2", [D], f32, isOutput=False)
    y = nc.declare_dram_parameter("y", [ROWS_PER_CORE, D], f32, isOutput=True)

    n_iter = N_BLOCKS * repeat

    with tile.TileContext(nc) as tc:
        with (
            tc.tile_pool(name="const", bufs=1) as const,
            tc.tile_pool(name="xin", bufs=2) as xin_pool,
            tc.tile_pool(name="xt", bufs=2) as xt_pool,
            tc.tile_pool(name="h", bufs=2) as h_pool,
            tc.tile_pool(name="out", bufs=2) as out_pool,
            tc.tile_pool(name="mask", bufs=2) as mask_pool,
            tc.tile_pool(name="stage", bufs=3, space="PSUM") as stage_pool,
            tc.tile_pool(name="py", bufs=4, space="PSUM") as py_pool,
            tc.tile_pool(name="warm", bufs=1, space="PSUM") as warm_pool,
        ):
            # Replicated parameters.  Chunked so the first matmuls can start
            # as soon as their slice arrives instead of behind 8 MB of DMA.
            w1s = const.tile([P, DC, F], f32r)  # [p, dc, f] <- w1[dc*128+p, f]
            w2s = const.tile([P, FC, D], f32r)  # [p, fc, d] <- w2[fc*128+p, d]
            b1s = const.tile([P, FC], f32)  # [p, fc] <- b1[fc*128+p]
            b2s = const.tile([P, D], f32)  # b2 replicated to all partitions
            ident = const.tile([P, P], f32r)

            w1r = w1.rearrange("(dc p) f -> p dc f", p=P).bitcast(f32r)
            w2r = w2.rearrange("(fc p) d -> p fc d", p=P).bitcast(f32r)

            def load_x(blk):
                rows = x[blk * R_BLOCK : (blk + 1) * R_BLOCK, :]
                src_ap = rows.rearrange("(rs p) d -> p rs d", p=P).bitcast(f32r)
                xn = xin_pool.tile([P, RT, D], f32r, name="xn")
                nc.sync.dma_start(xn[:], src_ap)
                return xn

            def mask_and_transpose(xn):
                # Transpose x -> xT [d_inner, dc, r] via PE (f32r: 1.5 c/row).
                # Emitted before the mask ops: the DVE psum->SBUF copies gate
                # the next mm1, the mask is only needed at the epilogue.
                xT = xt_pool.tile([P, DC, R_BLOCK], f32r, name="xT")
                for dc in range(DC):
                    # Grouped by d-chunk (not row-subtile): one DVE copy then
                    # delivers a complete mm1 rhs, so the first matmul starts
                    # as soon as the first chunk is staged.
                    pt = stage_pool.tile(
                        [P, RT, P], f32r, name="pt", tag="stage"
                    )
                    for rs in range(RT):
                        nc.tensor.transpose(
                            pt[:, rs, :],
                            xn[:, rs, dc * P : (dc + 1) * P],
                            ident[:],
                        )
                    last_copy = nc.vector.tensor_copy(xT[:, dc, :], pt[:])

                # Spike mask: 1.0 where max_d |x| > threshold else 0.0.
                amax = mask_pool.tile([P, RT], f32, name="amax")
                reduce_inst = nc.vector.tensor_reduce(
                    amax[:],
                    xn.bitcast(f32)[:],
                    axis=mybir.AxisListType.X,
                    op=mybir.AluOpType.max,
                    apply_absolute_value=True,
                )
                # Scheduling-only edge: keep the 2.2us reduce behind the xT
                # copies on DVE -- the copies gate the next mm1, the mask is
                # not needed until the epilogue.
                tile.add_dep_helper(
                    reduce_inst.ins, last_copy.ins, sync=False,
                    reason="mask reduce after xT copies",
                )
                mask = mask_pool.tile([P, RT], f32, name="mask")
                nc.vector.tensor_scalar(
                    mask[:], amax[:], THRESHOLD, None, op0=mybir.AluOpType.is_gt
                )
                return {"xT": xT, "mask": mask}

            # PE clock warm-up: the PE ramps to full clock only after ~3us
            # of sustained activity (HAM gate).  Burn the ramp on
            # dependency-free dummy matmuls (memset-fed, bf16) during the
            # dead window while x block 0 / w1 stream in, so the real
            # transposes+matmuls start at full rate.
            bf16 = mybir.dt.bfloat16
            wsrc = const.tile([P, D], bf16)
            nc.vector.memset(wsrc[:], 0.0)
            wdummy = warm_pool.tile([P, D], f32)
            for _ in range(10):
                nc.tensor.matmul(
                    wdummy[:], wsrc[:, 0:P], wsrc[:], start=True, stop=True
                )

            # --- startup: stream in first-use order.  Block 0's mm1 phase
            # only needs w1 (streamed in quarters just ahead of use); w2
            # chunks follow and land before block 0's (deferred) mm2 phase.
            xn0 = load_x(0)
            nc.sync.dma_start(w1s[:, :, 0:512], w1r[:, :, 0:512])
            nc.sync.dma_start(b1s[:], b1.rearrange("(fc p) -> p fc", p=P))
            nc.sync.dma_start(w1s[:, :, 512:1024], w1r[:, :, 512:1024])
            # Build identity in f32 scratch, then copy (=round) into the
            # f32r tile the transposes consume (BIR verifier requirement).
            ident_f32 = const.tile([P, P], f32)
            make_identity(nc, ident_f32)
            nc.vector.tensor_copy(ident[:], ident_f32[:])
            cur = mask_and_transpose(xn0)
            nc.sync.dma_start(w1s[:, :, 1024:1536], w1r[:, :, 1024:1536])
            nc.sync.dma_start(w1s[:, :, 1536:2048], w1r[:, :, 1536:2048])
            # x(1) right after w1 (its transposes run early in block 0's
            # deferred-mm2 phase), then w2 chunks just ahead of their mm2s.
            xn_next = load_x(1 % N_BLOCKS) if n_iter > 1 else None
            for wc in range(4):
                nc.sync.dma_start(
                    w2s[:, 4 * wc : 4 * (wc + 1), :],
                    w2r[:, 4 * wc : 4 * (wc + 1), :],
                )
            nc.sync.dma_start(b2s[:], b2[None, :].to_broadcast([P, D]))

            for it in range(n_iter):
                blk = it % N_BLOCKS
                xT, mask = cur["xT"], cur["mask"]

                hs = h_pool.tile([P, FC, R_BLOCK], f32r, name="hs")  # h^T
                psy = [
                    py_pool.tile([P, D], f32, name=f"psy{rt}", tag="psy")
                    for rt in range(RT)
                ]
                nxt = None
                # b2 * mask per row-subtile, off the critical path (feeds
                # the fused single-op epilogue below).
                yb = out_pool.tile([P, RT, D], f32, name="yb")
                for rt in range(RT):
                    nc.vector.tensor_scalar_mul(
                        yb[:, rt, :], b2s[:], mask[:, rt : rt + 1]
                    )

                def mm2(fc):
                    for rt in range(RT):
                        nc.tensor.matmul(
                            psy[rt][:],
                            hs[:, fc, rt * P : (rt + 1) * P],
                            w2s[:, fc, :],
                            start=(fc == 0),
                            stop=(fc == FC - 1),
                        )

                # Software-pipelined: mm2 runs one f-chunk behind mm1/relu
                # so PE never waits on ScalarE at block boundaries.  Block 0
                # instead defers ALL mm2s past the mm1 phase so they aren't
                # stuck behind the still-streaming w2 (PE does w1-only work
                # while w2 lands).
                mm2_lag = FC if (it == 0 or it == n_iter - 1) else 2
                for fc in range(FC):
                    ph = stage_pool.tile(
                        [P, R_BLOCK], f32, name="ph", tag="stage"
                    )
                    for dc in range(DC):
                        nc.tensor.matmul(
                            ph[:],
                            w1s[:, dc, fc * P : (fc + 1) * P],
                            xT[:, dc, :],
                            start=(dc == 0),
                            stop=(dc == DC - 1),
                        )
                    nc.scalar.activation(
                        hs[:, fc, :],
                        ph[:],
                        mybir.ActivationFunctionType.Relu,
                        bias=b1s[:, fc : fc + 1],
                    )
                    if fc >= mm2_lag:
                        mm2(fc - mm2_lag)
                    # Prefetch: x DMA for block it+2 early (fc==1), next
                    # block's transposes mid-way so PE never stalls.  For
                    # block 0 the transposes wait until fc==15 (x(1) is still
                    # behind w1 in the DMA stream at fc==7).
                    if fc == 1 and it + 2 < n_iter:
                        xn_next2 = load_x((it + 2) % N_BLOCKS)
                    if fc == (15 if it == 0 else 7) and it + 1 < n_iter:
                        nxt = mask_and_transpose(xn_next)
                        xn_next = xn_next2 if it + 2 < n_iter else None
                # Epilogue: yout = psy*mask + b2*mask, one fused DVE op per
                # row-subtile (psy bank freed after a single op).
                yout = out_pool.tile([P, RT, D], f32, name="yout")

                def epilogue(rt):
                    nc.vector.scalar_tensor_tensor(
                        yout[:, rt, :],
                        psy[rt][:],
                        mask[:, rt : rt + 1],
                        yb[:, rt, :],
                        op0=mybir.AluOpType.mult,
                        op1=mybir.AluOpType.add,
                    )
                    out_rows = y[
                        blk * R_BLOCK + rt * P : blk * R_BLOCK + (rt + 1) * P, :
                    ]
                    nc.sync.dma_start(out_rows, yout[:, rt, :])

                if it == n_iter - 1 and it != 0:
                    # Last block: group the remaining mm2s by row-subtile so
                    # each subtile's epilogue + store overlaps the next
                    # subtile's matmuls; only rt3's epilogue trails the final
                    # PE op before the kernel drain.
                    done = FC - mm2_lag
                    for rt in range(RT):
                        for fc in range(done, FC):
                            nc.tensor.matmul(
                                psy[rt][:],
                                hs[:, fc, rt * P : (rt + 1) * P],
                                w2s[:, fc, :],
                                start=(fc == 0),
                                stop=(fc == FC - 1),
                            )
                        epilogue(rt)
                else:
                    for fc in range(FC - mm2_lag, FC):
                        mm2(fc)
                    for rt in range(RT):
                        epilogue(rt)
                cur = nxt

    nc.finalize()
    return nc


def _get_program():
    if "nc" not in _CACHE:
        _CACHE["nc"] = _build_program()
    return _CACHE["nc"]


def kernel(x, w1, b1, w2, b2, _trace=False):
    from concourse.bass_utils import run_bass_kernel_spmd

    x = np.ascontiguousarray(np.asarray(x, dtype=np.float32))
    w1 = np.ascontiguousarray(np.asarray(w1, dtype=np.float32))
    b1 = np.ascontiguousarray(np.asarray(b1, dtype=np.float32))
    w2 = np.ascontiguousarray(np.asarray(w2, dtype=np.float32))
    b2 = np.ascontiguousarray(np.asarray(b2, dtype=np.float32))

    B, T, S, Dd = x.shape
    xf = x.reshape(-1, Dd)
    shards = np.split(xf, N_CORES, axis=0)
    in_maps = [
        {"x": s, "w1": w1, "b1": b1, "w2": w2, "b2": b2} for s in shards
    ]

    nc = _get_program()
    # The axon-tunneled devices occasionally throw a transient
    # NRT_EXEC_UNIT_UNRECOVERABLE; a fresh attempt succeeds.
    last_err = None
    for _attempt in range(3):
        try:
            res = run_bass_kernel_spmd(
                nc, in_maps, list(range(N_CORES)), trace=_trace
            )
            break
        except Exception as e:  # noqa: BLE001 - retry transient device faults
            last_err = e
            if "UNRECOVERABLE" not in str(e) and "UNAVAILABLE" not in str(e):
                raise
    else:
        raise last_err
    yf = np.concatenate([r["y"] for r in res.results], axis=0)
    out = yf.reshape(B, T, S, Dd).astype(np.float32)
    if _trace:
        return out, res
    return out



# revision 6
# speedup vs baseline: 1.1045x; 1.1045x over previous
"""Event-driven FFN kernel for Trainium2 (8 NeuronCores, data-parallel).

Reference computation (per row r of x[32768, 512]):
    mask[r] = any(|x[r, :]| > 0.01)
    y[r, :] = mask[r] * (relu(x[r, :] @ w1 + b1) @ w2 + b2)

Sharding: rows (B*T*S = 32768) split evenly across 8 cores; FFN weights
replicated.  Per core: 4096 rows, processed in 8 blocks of 512 rows.

Numerics: both matmuls run on the PE in fp8e4 (e4m3) DoubleRow perf mode,
which processes two 128-deep k-tiles per instruction at 0.5 cycles/row --
4x the f32r MACs/cycle.  Plain fp8 quantization costs ~5e-2 rel err, so
every matmul operand is split hi+lo (lo = exact residual of the hi
rounding, also e4m3): x @ w = x_hi w_hi + (x_hi w_lo + x_lo w_hi), dropping
the lo*lo term.  All three products share one PSUM accumulation because the
lo tensors are stored at natural scale (e4m3 subnormals cover them; the PE
handles fp8 subnormals exactly -- probed).  Global scales keep everything
in e4m3 range: x*4, w1*16, h*4, w2*16; the 1/64 comes out via the relu
activation scale (1/16) and the epilogue mask multiply (mask/64).
Measured end-to-end rel err vs the f32 reference: ~2e-3 (vs the 2e-2 gate).

Per-block dataflow on one core (512 rows per block):
  - DMA x block natural layout [128p, 4rs, 512d] f32
  - ScalarE: x_hi = fp8(x*4) into even bytes of a packed [p, rs, d, 2] tile
    DVE: x_lo = fp8(x*4 - x_hi) into odd bytes
  - 4x DMA-engine transpose (InstDmaTransposeAnt on the u16 pair view) ->
    xT [128d_in, 4dc, 512r] of (hi,lo) byte pairs.  No PE or DVE cycles.
  - mm1 per f-chunk (16): 6 DoubleRow matmuls into psum_h[f, r]:
    4x cross (ktiles x_hi[dc]*w1_lo[dc] + x_lo[dc]*w1_hi[dc]) + 2x hi*hi
    (ktiles dc pairs).  ScalarE relu (scale 1/16, bias 4*b1) -> hs32 f32;
    Pool copies hs32 -> h_hi fp8; DVE h_lo = fp8(hs32 - h_hi).
  - mm2 per fc-pair, two pairs behind mm1: 12 DoubleRow matmuls
    (per rt: cross fc, cross fc+1, hi*hi fc-pair) accumulate psum_y[rt].
  - mask: DVE abs-max reduce over xn -> is_gt threshold -> mask (1/0) and
    msc (mask/64); yb = b2*mask on ScalarE (activation Copy, scale=mask AP)
  - epilogue per rt: DVE yout = psy*msc + yb, DMA out
  - weights are pre-quantized host-side (hi/lo fp8, DoubleRow ktile layout)
    and streamed in first-use order; block 0 defers all mm2 past its mm1
    phase so it isn't gated on the still-streaming w2.

Engine budget per steady block (cost model): PE 20.9us (bottleneck),
DVE ~17, ScalarE ~15, Pool ~13.5, DMA ~8.  PE floor = 49152 cycles/block
= 3x the ideal fp8 DoubleRow pass (the compensation terms) = 0.75x f32r.
"""

import numpy as np

N_CORES = 8
ROWS_TOTAL = 32768  # 4 * 16 * 512
ROWS_PER_CORE = ROWS_TOTAL // N_CORES  # 4096
D = 512
F = 2048
R_BLOCK = 512
N_BLOCKS = ROWS_PER_CORE // R_BLOCK  # 8
P = 128
DC = D // P  # 4 d-chunks (mm1 k-tiles)
FC = F // P  # 16 f-chunks (mm2 k-tiles)
RT = R_BLOCK // P  # 4 row-subtiles per block
THRESHOLD = 0.01

SX = 4.0  # x pre-quant scale
SW = 16.0  # weight pre-quant scale
SH = 4.0  # h pre-quant scale
RELU_SCALE = SH / (SX * SW)  # 1/16: psum_h -> SH*h
OUT_SCALE = 1.0 / (SH * SW)  # 1/64: psum_y -> y (folded into mask)

_CACHE = {}


def _build_program():
    import concourse.mybir as mybir
    import concourse.tile as tile
    from concourse import bacc

    f32 = mybir.dt.float32
    fp8 = mybir.dt.float8e4
    u16 = mybir.dt.uint16
    bf16 = mybir.dt.bfloat16
    DR = mybir.MatmulPerfMode.DoubleRow
    Relu = mybir.ActivationFunctionType.Relu
    Copy = mybir.ActivationFunctionType.Copy

    nc = bacc.Bacc()

    x = nc.declare_dram_parameter("x", [ROWS_PER_CORE, D], f32, isOutput=False)
    # Host-prequantized weights (hi/lo fp8 in DoubleRow k-tile layouts).
    w1hi = nc.declare_dram_parameter("w1hi", [P, DC, F], fp8, isOutput=False)
    w1x = nc.declare_dram_parameter("w1x", [P, DC, 2, F], fp8, isOutput=False)
    w2hi = nc.declare_dram_parameter("w2hi", [P, FC, D], fp8, isOutput=False)
    w2x = nc.declare_dram_parameter("w2x", [P, FC, 2, D], fp8, isOutput=False)
    b1s = nc.declare_dram_parameter("b1s", [P, FC], f32, isOutput=False)  # SH*b1
    b2 = nc.declare_dram_parameter("b2", [D], f32, isOutput=False)
    y = nc.declare_dram_parameter("y", [ROWS_PER_CORE, D], f32, isOutput=True)

    n_iter = N_BLOCKS

    with tile.TileContext(nc) as tc:
        with (
            tc.tile_pool(name="const", bufs=1) as const,
            tc.tile_pool(name="xin", bufs=2) as xin_pool,
            tc.tile_pool(name="xq", bufs=2) as xq_pool,
            tc.tile_pool(name="xt", bufs=2) as xt_pool,
            tc.tile_pool(name="h", bufs=2) as h_pool,
            tc.tile_pool(name="h32", bufs=4) as h32_pool,
            tc.tile_pool(name="out", bufs=2) as out_pool,
            tc.tile_pool(name="mask", bufs=2) as mask_pool,
            tc.tile_pool(name="stage", bufs=3, space="PSUM") as stage_pool,
            tc.tile_pool(name="py", bufs=4, space="PSUM") as py_pool,
            tc.tile_pool(name="warm", bufs=1, space="PSUM") as warm_pool,
        ):
            w1hi_s = const.tile([P, DC, F], fp8)
            w1x_s = const.tile([P, DC, 2, F], fp8)  # [:, dc, 0]=lo, [:, dc, 1]=hi
            w2hi_s = const.tile([P, FC, D], fp8)
            w2x_s = const.tile([P, FC, 2, D], fp8)
            b1_s = const.tile([P, FC], f32)
            b2_s = const.tile([P, D], f32)

            def load_x(blk):
                rows = x[blk * R_BLOCK : (blk + 1) * R_BLOCK, :]
                src_ap = rows.rearrange("(rs p) d -> p rs d", p=P)
                xn = xin_pool.tile([P, RT, D], f32, name="xn")
                nc.sync.dma_start(xn[:], src_ap)
                return xn

            def quant_transpose(xn):
                """fp8 hi/lo quantize + DMA-engine transpose + spike mask."""
                xqp = xq_pool.tile([P, RT, D, 2], fp8, name="xqp")
                # hi into even bytes (ScalarE), lo residual into odd (DVE)
                nc.scalar.activation(xqp[:, :, :, 0], xn[:], Copy, scale=SX)
                xlo_inst = nc.vector.scalar_tensor_tensor(
                    xqp[:, :, :, 1], xn[:], SX, xqp[:, :, :, 0],
                    op0=mybir.AluOpType.mult, op1=mybir.AluOpType.subtract,
                )
                # xT[d_in, dc, r] of u16 (hi,lo) pairs; one DMA transpose
                # per row-subtile: out[p, dc, r] = in[r, 128*dc + p]
                xT = xt_pool.tile([P, DC, R_BLOCK], u16, name="xT")
                for rs in range(RT):
                    nc.sync.dma_start_transpose(
                        xT[:, :, rs * P : (rs + 1) * P],
                        xqp[:, rs, :, :].bitcast(u16),
                    )
                # Spike mask: max_d |x| > thr -> mask (1/0) and msc (mask/64)
                amax = mask_pool.tile([P, RT], f32, name="amax")
                reduce_inst = nc.vector.tensor_reduce(
                    amax[:], xn[:], axis=mybir.AxisListType.X,
                    op=mybir.AluOpType.max, apply_absolute_value=True,
                )
                tile.add_dep_helper(
                    reduce_inst.ins, xlo_inst.ins, sync=False,
                    reason="mask reduce after x_lo",
                )
                mask = mask_pool.tile([P, RT], f32, name="mask")
                nc.vector.tensor_scalar(
                    mask[:], amax[:], THRESHOLD, None, op0=mybir.AluOpType.is_gt
                )
                msc = mask_pool.tile([P, RT], f32, name="msc")
                nc.vector.tensor_scalar(
                    msc[:], amax[:], THRESHOLD, OUT_SCALE,
                    op0=mybir.AluOpType.is_gt, op1=mybir.AluOpType.mult,
                )
                # fp8 views for the matmuls:
                #   cross rhs per dc:  [p, 2{hi,lo}, 512r]   strides (1, 2)
                #   hihi  rhs per t:   [p, 2{dc pair}, 512r] strides (1024, 2)
                xTf = xT[:].bitcast(fp8).rearrange("p a (r t) -> p a r t", t=2)
                return {"xTf": xTf, "mask": mask, "msc": msc}

            # PE clock warm-up (HAM gate ramps over ~3us): dependency-free
            # dummy matmuls while the first x block and w1 stream in.
            wsrc = const.tile([P, D], bf16)
            nc.vector.memset(wsrc[:], 0.0)
            wdummy = warm_pool.tile([P, D], f32)
            for _ in range(10):
                nc.tensor.matmul(
                    wdummy[:], wsrc[:, 0:P], wsrc[:], start=True, stop=True
                )

            # --- startup stream, first-use order.  The DMA transposes of
            # block 0 are SP-issued between the first and second weight
            # chunks so they don't queue behind the whole weight stream.
            xn0 = load_x(0)
            FQ = F // 4
            nc.sync.dma_start(w1hi_s[:, :, 0:FQ], w1hi[:, :, 0:FQ])
            nc.sync.dma_start(w1x_s[:, :, :, 0:FQ], w1x[:, :, :, 0:FQ])
            nc.sync.dma_start(b1_s[:], b1s[:, :])
            nc.sync.dma_start(b2_s[:], b2[None, :].to_broadcast([P, D]))

            # block 0 quantize + transpose (SP SEQ stalls on x_lo, then the
            # 4 transposes go ahead of the remaining weight chunks)
            xqp0 = xq_pool.tile([P, RT, D, 2], fp8, name="xqp")
            nc.scalar.activation(xqp0[:, :, :, 0], xn0[:], Copy, scale=SX)
            lv = nc.vector.scalar_tensor_tensor(
                xqp0[:, :, :, 1], xn0[:], SX, xqp0[:, :, :, 0],
                op0=mybir.AluOpType.mult, op1=mybir.AluOpType.subtract,
            )
            xT0 = xt_pool.tile([P, DC, R_BLOCK], u16, name="xT")
            for rs in range(RT):
                nc.sync.dma_start_transpose(
                    xT0[:, :, rs * P : (rs + 1) * P],
                    xqp0[:, rs, :, :].bitcast(u16),
                )
            amax0 = mask_pool.tile([P, RT], f32, name="amax")
            ri = nc.vector.tensor_reduce(
                amax0[:], xn0[:], axis=mybir.AxisListType.X,
                op=mybir.AluOpType.max, apply_absolute_value=True,
            )
            tile.add_dep_helper(ri.ins, lv.ins, sync=False, reason="mask last")
            mask0 = mask_pool.tile([P, RT], f32, name="mask")
            nc.vector.tensor_scalar(
                mask0[:], amax0[:], THRESHOLD, None, op0=mybir.AluOpType.is_gt
            )
            msc0 = mask_pool.tile([P, RT], f32, name="msc")
            nc.vector.tensor_scalar(
                msc0[:], amax0[:], THRESHOLD, OUT_SCALE,
                op0=mybir.AluOpType.is_gt, op1=mybir.AluOpType.mult,
            )
            xTf0 = xT0[:].bitcast(fp8).rearrange("p a (r t) -> p a r t", t=2)
            cur = {"xTf": xTf0, "mask": mask0, "msc": msc0}

            # remaining weights + next x block
            for c in range(1, 4):
                nc.sync.dma_start(
                    w1hi_s[:, :, c * FQ : (c + 1) * FQ],
                    w1hi[:, :, c * FQ : (c + 1) * FQ],
                )
                nc.sync.dma_start(
                    w1x_s[:, :, :, c * FQ : (c + 1) * FQ],
                    w1x[:, :, :, c * FQ : (c + 1) * FQ],
                )
            xn_next = load_x(1) if n_iter > 1 else None
            for c in range(4):
                nc.sync.dma_start(
                    w2hi_s[:, 4 * c : 4 * (c + 1), :],
                    w2hi[:, 4 * c : 4 * (c + 1), :],
                )
                nc.sync.dma_start(
                    w2x_s[:, 4 * c : 4 * (c + 1), :, :],
                    w2x[:, 4 * c : 4 * (c + 1), :, :],
                )

            for it in range(n_iter):
                blk = it
                xTf, mask, msc = cur["xTf"], cur["mask"], cur["msc"]

                # h tile: [p, fc, {hi,lo}, r] fp8
                hs = h_pool.tile([P, FC, 2, R_BLOCK], fp8, name="hs")
                psy = [
                    py_pool.tile([P, D], f32, name=f"psy{rt}", tag="psy")
                    for rt in range(RT)
                ]
                nxt = None
                # b2*mask per row-subtile, off the critical path (ScalarE
                # activation Copy with per-partition scale = mask column).
                yb = out_pool.tile([P, RT, D], f32, name="yb")
                for rt in range(RT):
                    nc.scalar.activation(
                        yb[:, rt, :], b2_s[:], Copy,
                        scale=mask[:, rt : rt + 1],
                    )

                def mm1(fc, ph):
                    fsl = slice(fc * P, (fc + 1) * P)
                    for dc in range(DC):
                        nc.tensor.matmul(
                            ph[:],
                            w1x_s[:, dc, :, fsl],
                            xTf[:, dc, :, :].rearrange("p r t -> p t r"),
                            start=(dc == 0), stop=False, perf_mode=DR,
                        )
                    for t in range(2):
                        nc.tensor.matmul(
                            ph[:],
                            w1hi_s[:, 2 * t : 2 * t + 2, fsl],
                            xTf[:, 2 * t : 2 * t + 2, :, 0],
                            start=False, stop=(t == 1), perf_mode=DR,
                        )

                def mm2_pair(t, rts=range(RT)):
                    for rt in rts:
                        rsl = slice(rt * P, (rt + 1) * P)
                        nc.tensor.matmul(
                            psy[rt][:], hs[:, 2 * t, :, rsl],
                            w2x_s[:, 2 * t, :, :],
                            start=(t == 0), stop=False, perf_mode=DR,
                        )
                        nc.tensor.matmul(
                            psy[rt][:], hs[:, 2 * t + 1, :, rsl],
                            w2x_s[:, 2 * t + 1, :, :],
                            start=False, stop=False, perf_mode=DR,
                        )
                        nc.tensor.matmul(
                            psy[rt][:], hs[:, 2 * t : 2 * t + 2, 0, rsl],
                            w2hi_s[:, 2 * t : 2 * t + 2, :],
                            start=False, stop=(t == FC // 2 - 1), perf_mode=DR,
                        )

                # mm2 lags mm1 by `lag` fc-pairs; block 0 defers all mm2
                # past its mm1 phase (w2 still streaming).
                lag = FC // 2 if (it == 0 or it == n_iter - 1) else 2
                for fc in range(FC):
                    ph = stage_pool.tile([P, R_BLOCK], f32, name="ph",
                                         tag="stage")
                    mm1(fc, ph)
                    hs32 = h32_pool.tile([P, R_BLOCK], f32, name="hs32")
                    nc.scalar.activation(
                        hs32[:], ph[:], Relu,
                        bias=b1_s[:, fc : fc + 1], scale=RELU_SCALE,
                    )
                    nc.gpsimd.tensor_copy(hs[:, fc, 0, :], hs32[:])
                    nc.vector.scalar_tensor_tensor(
                        hs[:, fc, 1, :], hs32[:], 1.0, hs[:, fc, 0, :],
                        op0=mybir.AluOpType.mult, op1=mybir.AluOpType.subtract,
                    )
                    if fc % 2 == 1:
                        t = (fc - 1) // 2 - lag
                        if t >= 0:
                            mm2_pair(t)
                    if fc == 1 and it + 2 < n_iter:
                        xn_next2 = load_x(it + 2)
                    if fc == (15 if it == 0 else 7) and it + 1 < n_iter:
                        nxt = quant_transpose(xn_next)
                        xn_next = xn_next2 if it + 2 < n_iter else None

                yout = out_pool.tile([P, RT, D], f32, name="yout")

                def epilogue(rt):
                    nc.vector.scalar_tensor_tensor(
                        yout[:, rt, :], psy[rt][:], msc[:, rt : rt + 1],
                        yb[:, rt, :],
                        op0=mybir.AluOpType.mult, op1=mybir.AluOpType.add,
                    )
                    out_rows = y[
                        blk * R_BLOCK + rt * P : blk * R_BLOCK + (rt + 1) * P, :
                    ]
                    nc.sync.dma_start(out_rows, yout[:, rt, :])

                done = FC // 2 - lag
                if it == n_iter - 1 and it != 0:
                    # Last block: group remaining mm2 by row-subtile so each
                    # subtile's epilogue+store overlaps the next subtile's
                    # matmuls; only rt3's epilogue trails the final PE op.
                    for rt in range(RT):
                        for t in range(done, FC // 2):
                            mm2_pair(t, rts=[rt])
                        epilogue(rt)
                else:
                    for t in range(done, FC // 2):
                        mm2_pair(t)
                    for rt in range(RT):
                        epilogue(rt)
                cur = nxt

    nc.finalize()
    return nc


def _get_program():
    if "nc" not in _CACHE:
        _CACHE["nc"] = _build_program()
    return _CACHE["nc"]


def _quant_weights(w1, b1, w2, b2):
    """Host-side hi/lo e4m3 pre-quantization + DoubleRow k-tile layouts."""
    import ml_dtypes

    E4 = ml_dtypes.float8_e4m3

    def hilo(w):
        ws = (w * SW).astype(np.float32)
        hi = ws.astype(E4)
        lo = (ws - hi.astype(np.float32)).astype(E4)
        return hi, lo

    w1h, w1l = hilo(w1)  # [D, F]
    w1hi = np.ascontiguousarray(
        w1h.reshape(DC, P, F).transpose(1, 0, 2))  # [P, DC, F]
    w1lo = w1l.reshape(DC, P, F).transpose(1, 0, 2)
    w1x = np.ascontiguousarray(
        np.stack([w1lo, w1hi], axis=2))  # [P, DC, 2{lo,hi}, F]

    w2h, w2l = hilo(w2)  # [F, D]
    w2hi = np.ascontiguousarray(
        w2h.reshape(FC, P, D).transpose(1, 0, 2))  # [P, FC, D]
    w2lo = w2l.reshape(FC, P, D).transpose(1, 0, 2)
    w2x = np.ascontiguousarray(
        np.stack([w2lo, w2hi], axis=2))  # [P, FC, 2{lo,hi}, D]

    b1s = np.ascontiguousarray(
        (SH * b1).reshape(FC, P).T).astype(np.float32)  # [P, FC]
    return {
        "w1hi": w1hi, "w1x": w1x, "w2hi": w2hi, "w2x": w2x,
        "b1s": b1s, "b2": np.ascontiguousarray(b2, dtype=np.float32),
    }


def kernel(x, w1, b1, w2, b2, _trace=False):
    from concourse.bass_utils import run_bass_kernel_spmd

    x = np.ascontiguousarray(np.asarray(x, dtype=np.float32))
    w1 = np.ascontiguousarray(np.asarray(w1, dtype=np.float32))
    b1 = np.ascontiguousarray(np.asarray(b1, dtype=np.float32))
    w2 = np.ascontiguousarray(np.asarray(w2, dtype=np.float32))
    b2 = np.ascontiguousarray(np.asarray(b2, dtype=np.float32))

    B, T, S, Dd = x.shape
    xf = x.reshape(-1, Dd)
    shards = np.split(xf, N_CORES, axis=0)
    wmap = _quant_weights(w1, b1, w2, b2)
    in_maps = [{"x": s, **wmap} for s in shards]

    nc = _get_program()
    # The axon-tunneled devices occasionally throw a transient
    # NRT_EXEC_UNIT_UNRECOVERABLE; a fresh attempt succeeds.
    last_err = None
    for _attempt in range(3):
        try:
            res = run_bass_kernel_spmd(
                nc, in_maps, list(range(N_CORES)), trace=_trace
            )
            break
        except Exception as e:  # noqa: BLE001 - retry transient device faults
            last_err = e
            if "UNRECOVERABLE" not in str(e) and "UNAVAILABLE" not in str(e):
                raise
    else:
        raise last_err
    yf = np.concatenate([r["y"] for r in res.results], axis=0)
    out = yf.reshape(B, T, S, Dd).astype(np.float32)
    if _trace:
        return out, res
    return out
